# revision 23
# baseline (speedup 1.0000x reference)
"""Trainium2 Bass kernel for nn_LSTMModel (B=2048, T=512, I=1, H=64, O=1).

Strategy: pure data parallel over 8 NeuronCores (256 batch rows each).
Within a core, 3 staggered groups (86/85/85 batch columns) run the T=512
recurrence as software-pipelined independent chains.  The per-step
dependency cycle (MM -> sigmoid -> cell update -> tanh -> h -> MM) is
latency-bound, so group width is chosen to shorten each op on the cycle
while keeping total ACT-engine busy below the cycle latency.

Per group and step, gates are computed with gate-pairs stacked on 128
partitions (stationary [f|g] and [o|i]), so ONE sigmoid activation over
[128, 2*Wg] covers all four gates.  State/gates/weights are fp16
(1 matmul cycle/row vs fp32's 4; packed 2-byte = 2x DVE mode); the cell
state c is also fp16 (all-fp16 TT add gets the 2x DVE mode).

Partition-base choreography (verifier requires equal bases for SBUF
input pairs; outputs may shift):
  f^ @0, g^ @64 (pair 1);  o^ @0, i^ @64 (pair 2)
  q = (g^@64 - 0.5) * i^@64      -> q@0      (STT on DVE)
  r = f^@0 * c@0                 -> r@0      (TT on gpsimd, parallel with q)
  c = q@0 + r@0                  -> c@0      (TT, in place)
  tau = tanh(2*c)@0              -> tau@0    (ACT)
  h = tau@0 * o^@0               -> state rows 0:64  (TT)

Math (weights pre-scaled on host, c~ = c/2, h stored at full scale):
  g-gate preactivation doubled so sigmoid(2a) = (tanh(a)+1)/2 serves it
  inside the big sigmoid op; x_t and the bias ride the recurrent matmul
  as two extra K rows.  x rows are DMA'd two steps per transfer into a
  single ring tile, ~3 steps ahead (emitted after the matmuls that last
  read the target ring slots, so the WAR ordering is correct).

Anti-phase seeding: groups 1 and 2 get an artificial zero-valued
dependency on group 0's first sigmoid / cell-add outputs so the three
chains settle ~1/3 period apart instead of locksteppping.
"""

import numpy as np

B, T, I, H, O = 2048, 512, 1, 64, 1
NCORES = 8
BC = B // NCORES            # 256 batch rows per core
GROUPS = [86, 85, 85]       # batch columns per group (sum = BC)
NG = len(GROUPS)
NB = 4                      # state ring buffers per group (even, for x pairs)
K = H + 2                   # h rows + ones row + x row

_CACHE = {}


def _build_program(T=T):
    import concourse.bacc as bacc
    import concourse.tile as tile
    from concourse import mybir

    f32 = mybir.dt.float32
    bf16 = mybir.dt.float16   # 2-byte dtype for state/gates/weights
    AF = mybir.ActivationFunctionType
    OP = mybir.AluOpType

    nc = bacc.Bacc("TRN2", target_bir_lowering=False, debug=False)

    w1_d = nc.dram_tensor("w1", (K, 128), bf16, kind="ExternalInput").ap()
    w2_d = nc.dram_tensor("w2", (K, 128), bf16, kind="ExternalInput").ap()
    wout_d = nc.dram_tensor("wout", (K, 1), bf16, kind="ExternalInput").ap()
    xt_d = [
        nc.dram_tensor(f"xt{g}", (T // 2, 2 * wg), bf16, kind="ExternalInput").ap()
        for g, wg in enumerate(GROUPS)
    ]
    y_d = nc.dram_tensor("y", (1, BC), f32, kind="ExternalOutput").ap()

    with tile.TileContext(nc) as tc:
        with (
            tc.tile_pool(name="consts", bufs=1) as wpool,
            tc.tile_pool(name="state", bufs=1) as spool,
            tc.tile_pool(name="gates", bufs=2) as gpool,
            tc.tile_pool(name="tmp", bufs=3) as tpool,
            tc.tile_pool(name="psum", bufs=1, space="PSUM") as pspool,
            tc.tile_pool(name="opsum", bufs=1, space="PSUM") as opspool,
        ):
            w1 = wpool.tile([K, 128], bf16, tag="w1")
            w2 = wpool.tile([K, 128], bf16, tag="w2")
            wo = wpool.tile([K, 1], bf16, tag="wo")
            zz = wpool.tile([H, 128], bf16, tag="zz")   # zeros, for phase seeds
            nc.sync.dma_start(w1[:], w1_d[:])
            nc.sync.dma_start(w2[:], w2_d[:])
            nc.sync.dma_start(wo[:], wout_d[:])
            nc.vector.memset(zz[:], 0.0)

            cst = []    # cell state per group [H, Wg] fp16
            srng = []   # state ring per group: one [K, NB*Wg] tile
            for g, wg in enumerate(GROUPS):
                c = spool.tile([H, wg], bf16, tag=f"c{g}")
                nc.vector.memset(c[:], 0.0)
                cst.append(c)
                s = spool.tile([K, NB * wg], bf16, tag=f"s{g}")
                nc.vector.memset(s[0:H, :], 0.0)
                nc.vector.memset(s[H : H + 1, :], 1.0)
                srng.append(s)

            # pre-issue x pair-DMAs for steps 0..3 (ring slots 0..3)
            for g, wg in enumerate(GROUPS):
                for p in range(2):
                    nc.sync.dma_start(
                        srng[g][H + 1 : K, p * 2 * wg : (p + 1) * 2 * wg],
                        xt_d[g][p : p + 1, :],
                    )

            seed_src = [None] * max(1, NG - 1)  # group0 step0 tiles, by delay

            for t in range(T):
                for g, wg in enumerate(GROUPS):
                    s = srng[g]
                    c = cst[g]
                    sb = s[:, (t % NB) * wg : (t % NB + 1) * wg]
                    # two M=128 matmuls: [f|g] and [o|i] gate pairs
                    ps = pspool.tile([128, 2 * wg], f32, tag=f"ps{g}")
                    nc.tensor.matmul(
                        ps[:, 0:wg], w1[:], sb, start=True, stop=True
                    )
                    nc.tensor.matmul(
                        ps[:, wg : 2 * wg], w2[:], sb, start=True, stop=True
                    )
                    # one sigmoid for all 4 gates: [128, 2*Wg]
                    gt = gpool.tile([128, 2 * wg], bf16, tag=f"g{g}")
                    nc.scalar.activation(gt[:], ps[:], AF.Sigmoid)
                    fh = gt[0:H, 0:wg]            # f^ @0
                    gh = gt[H:128, 0:wg]          # g^ @64
                    oh = gt[0:H, wg : 2 * wg]     # o^ @0
                    ih = gt[H:128, wg : 2 * wg]   # i^ @64
                    # q~ = (g^ - 0.5) * i^   (inputs @64 -> out @0)
                    q = tpool.tile([H, wg], bf16, tag=f"q{g}")
                    nc.vector.scalar_tensor_tensor(
                        q[:], gh, 0.5, ih, OP.subtract, OP.mult
                    )
                    # r = f^ * c~  on gpsimd, concurrent with q on DVE
                    r = tpool.tile([H, wg], bf16, tag=f"r{g}")
                    nc.gpsimd.tensor_mul(r[:], fh, c[:])
                    # c~ = q~ + r (in place)
                    nc.vector.tensor_add(c[:], q[:], r[:])
                    # tau = tanh(2 c~) = tanh(c)
                    tch = tpool.tile([H, wg], bf16, tag=f"t{g}")
                    nc.scalar.activation(tch[:], c[:], AF.Tanh, scale=2.0)
                    # h = tau * o^ -> next state buffer's h rows
                    nxt = s[0:H, ((t + 1) % NB) * wg : ((t + 1) % NB + 1) * wg]
                    nc.vector.tensor_mul(nxt, tch[:], oh)

                    # paired x prefetch: at odd t (after this step's MMs are
                    # emitted), fetch steps t+3, t+4 into the two ring slots
                    # just consumed by MM(t-1) and MM(t)
                    if t % 2 == 1 and t + 3 < T:
                        p = (t + 3) // 2
                        col = ((t + 3) % NB) * wg
                        nc.sync.dma_start(
                            s[H + 1 : K, col : col + 2 * wg],
                            xt_d[g][p : p + 1, :],
                        )

                    # anti-phase seeds, once, after group 0's step 0:
                    # group k+1 gets a zero-valued dep on a progressively
                    # later group-0 tile (sigmoid out, cell add, tanh out)
                    if t == 0 and g == 0:
                        for k in range(NG - 1):
                            seed_src[k] = (gt, c, tch)[min(k, 2)]
                    if t == 0 and g < NG - 1:
                        wgn = GROUPS[g + 1]
                        src = seed_src[g][0:H, 0:wgn]
                        nc.vector.tensor_mul(
                            srng[g + 1][0:H, 0:wgn], src, zz[:, 0:wgn]
                        )

            col0 = 0
            for g, wg in enumerate(GROUPS):
                sb = srng[g][:, (T % NB) * wg : (T % NB + 1) * wg]
                pso = opspool.tile([1, wg], f32, tag="po")
                nc.tensor.matmul(pso[:], wo[:], sb, start=True, stop=True)
                yt = tpool.tile([1, wg], f32, tag=f"y{g}")
                nc.vector.tensor_copy(yt[:], pso[:])
                nc.sync.dma_start(y_d[0:1, col0 : col0 + wg], yt[:])
                col0 += wg

    nc.compile()
    return nc


def _prep_weights(w_ih, w_hh, b_ih, b_hh, w_lin, b_lin):
    """Host-side pre-scaled stationary operands ([K, M] = lhsT layout)."""
    w_hh = np.asarray(w_hh, np.float32)
    w_ih = np.asarray(w_ih, np.float32)
    bias = np.asarray(b_ih, np.float32) + np.asarray(b_hh, np.float32)
    # per-gate-row scale: 1 for i,f,o; 2 for g (sigmoid(2a) trick)
    s = np.ones((4 * H, 1), np.float32)
    s[2 * H : 3 * H] = 2.0
    # state rows: [h (0:64, FULL scale); ones (64); x (65)]
    waug = np.concatenate(
        [s * w_hh, s * bias[:, None], s * w_ih[:, :1]], axis=1
    )  # [4H, K]
    i_r = slice(0, H)
    f_r = slice(H, 2 * H)
    g_r = slice(2 * H, 3 * H)
    o_r = slice(3 * H, 4 * H)
    # lhsT col-blocks: w1 = [f | g], w2 = [o | i]
    w1 = np.ascontiguousarray(
        np.concatenate([waug[f_r], waug[g_r]], axis=0).T
    ).astype(np.float16)  # [K, 128]
    w2 = np.ascontiguousarray(
        np.concatenate([waug[o_r], waug[i_r]], axis=0).T
    ).astype(np.float16)  # [K, 128]
    wout = np.zeros((K, 1), np.float32)
    wout[0:H, 0] = np.asarray(w_lin, np.float32)[0]
    wout[H, 0] = float(np.asarray(b_lin, np.float32)[0])
    return w1, w2, wout.astype(np.float16)


def _make_in_maps(x, w_ih, w_hh, b_ih, b_hh, w_lin, b_lin):
    w1, w2, wout = _prep_weights(w_ih, w_hh, b_ih, b_hh, w_lin, b_lin)
    x = np.asarray(x, np.float32).reshape(B, T)  # I == 1
    in_maps = []
    for core in range(NCORES):
        xc = x[core * BC : (core + 1) * BC]  # [BC, T]
        m = {"w1": w1, "w2": w2, "wout": wout}
        g0 = 0
        for g, wg in enumerate(GROUPS):
            xg = xc[g0 : g0 + wg]  # [Wg, T]
            g0 += wg
            # [T, Wg] -> pairs of steps side by side: [T//2, 2*Wg]
            xt = np.ascontiguousarray(xg.T).reshape(T // 2, 2 * wg)
            m[f"xt{g}"] = xt.astype(np.float16)
        in_maps.append(m)
    return in_maps


def kernel(x, w_ih, w_hh, b_ih, b_hh, w_lin, b_lin):
    from concourse import bass_utils

    if "nc" not in _CACHE:
        _CACHE["nc"] = _build_program()
    nc = _CACHE["nc"]

    in_maps = _make_in_maps(x, w_ih, w_hh, b_ih, b_hh, w_lin, b_lin)
    res = bass_utils.run_bass_kernel_spmd(
        nc, in_maps, core_ids=list(range(NCORES))
    )
    out = np.concatenate(
        [r["y"].reshape(-1) for r in res.results]
    )  # [B] in batch order
    return out.reshape(B, O).astype(np.float32)
